# revision 12
# baseline (speedup 1.0000x reference)
"""Multi-head causal self-attention on 8 Trainium2 NeuronCores.

Tensor-parallel over heads: core i owns heads (2i, 2i+1).
Per core:
  phase 1: qT/kT/vT = (W_slice^T @ x^T) for its 2 heads (xT provided by host);
           vT transposed on PE into [token, d] tiles for both heads at once.
  phase 2: per (b, h), k-chunk-outer loop sharing each stationary operand
           across all valid q-blocks: scoresT[k,q] = K Q^T -> exp ->
           (causal mask) -> out[d+1, q] accumulated as [V | 1]^T @ attnT
           (extra row = softmax denominator); normalize via 1/l broadcast.
  phase 3: partial projection P_i = W_proj[own rows]^T @ A_i, chunked over
           token ranges; ReduceScatter(add) each chunk across the 8 cores
           (overlaps with remaining attention compute); + bias.
Host reassembles the 8 column slices.

Matmuls run as float32r (fp32 storage, fast PE mode): ~230 ns per 512-col
matmul vs 429 ns for fp32, at ~1e-4 component relative error.
"""

import os

import numpy as np

B, T, C, H = 2, 2048, 1024, 16
D = C // H            # 64
NCORES = 8
HL = H // NCORES      # 2 heads per core
NT = B * T            # 4096
NQ = T // 512         # q-blocks of 512 per (b,h)
NK = T // 128         # k-chunks of 128 per (b,h)
SCALE = float(D) ** -0.5

MM_FAST = True        # float32r matmuls vs float32

_cache = {}


def _build(mode: str):
    """mode: 'causal' | 'none' (all-ones mask)."""
    import concourse.mybir as mybir
    import concourse.tile as tile
    from concourse import bacc

    f32 = mybir.dt.float32
    mdt = mybir.dt.float32r if MM_FAST else f32

    nc = bacc.Bacc("TRN2", target_bir_lowering=False, debug=False,
                   num_devices=NCORES)
    xT = nc.dram_tensor("xT", [C, NT], mdt, kind="ExternalInput").ap()
    wqkv = nc.dram_tensor("wqkv", [C, 3 * HL * D], mdt,
                          kind="ExternalInput").ap()
    wp = nc.dram_tensor("wp", [128, C], mdt, kind="ExternalInput").ap()
    bias = nc.dram_tensor("bias", [128, 1], f32, kind="ExternalInput").ap()
    cmask = nc.dram_tensor("cmask", [128, 4 * 512 + 128], f32,
                           kind="ExternalInput").ap()
    ones_in = nc.dram_tensor("ones_in", [128, 64], mdt,
                             kind="ExternalInput").ap()
    outT = nc.dram_tensor("outT", [128, NT], f32, kind="ExternalOutput").ap()

    causal = mode == "causal"
    Exp = mybir.ActivationFunctionType.Exp

    with tile.TileContext(nc) as tc, \
         nc.allow_low_precision(reason="float32r matmul fast path"):
        with tc.tile_pool(name="persist", bufs=1) as persist, \
             tc.tile_pool(name="dram", bufs=1, space="DRAM") as dram:
            q_sb = persist.tile([128, NT], mdt)
            k_sb = persist.tile([128, NT], mdt)
            # V tiles, both heads, each with a trailing ones column:
            # cols 0:64 = head0 d, col 64 = ones, 65:129 = head1 d, 129 = ones
            vboth = persist.tile([128, 131, B * NK], mdt)
            a_sb = persist.tile([128, NT], mdt)   # normalized attn out
            cm_sb = persist.tile([128, 4 * 512 + 128], f32)
            ones_sb = persist.tile([1, 64], mdt)
            wqkv_sb = persist.tile([128, 8, 3 * HL * D], mdt)
            wp_sb = persist.tile([128, C], mdt)
            bias_sb = persist.tile([128, 1], f32)
            prt = dram.tile([2 * B, C, 1024], f32)      # proj partials
            rso = dram.tile([2 * B, 128, 1024], f32)    # RS outputs

            nc.sync.dma_start(out=cm_sb[:], in_=cmask[:])
            nc.sync.dma_start(out=wqkv_sb[:],
                              in_=wqkv.rearrange("(a p) n -> p a n", p=128))
            nc.sync.dma_start(out=wp_sb[:], in_=wp[:])
            nc.sync.dma_start(out=bias_sb[:], in_=bias[:])
            nc.sync.dma_start(out=ones_sb[:], in_=ones_in[0:1, :])
            nc.sync.dma_start(out=vboth[:, 64, :], in_=ones_in[:, 0:B * NK])
            nc.sync.dma_start(out=vboth[:, 130 - 1, :],
                              in_=ones_in[:, 0:B * NK])
            ident = cm_sb[:, 4 * 512:4 * 512 + 128]

            # ---- phase 1: qkvT = W_slice^T @ xT ----
            with tc.tile_pool(name="xn_pool", bufs=5) as xp, \
                 tc.tile_pool(name="qkv_psum", bufs=4, space="PSUM") as pp, \
                 tc.tile_pool(name="vt_psum", bufs=2, space="PSUM") as vtp, \
                 tc.tile_pool(name="vtmp_pool", bufs=2) as vpool:
                for ng in range(2):
                    xns = []
                    for k in range(4):
                        n = ng * 4 + k
                        xn = xp.tile([128, 8, 512], mdt, tag="xn")
                        nc.sync.dma_start(
                            out=xn[:],
                            in_=xT.rearrange("(a p) t -> p a t", p=128)
                            [:, :, n * 512:(n + 1) * 512])
                        xns.append((n, xn))
                    for m in range(3):  # 0: q, 1: k, 2: v
                        pss = [pp.tile([128, 512], f32, tag="qkv", name="qkvp")
                               for _ in range(4)]
                        for kc in range(8):
                            for idx, (n, xn) in enumerate(xns):
                                nc.tensor.matmul(
                                    pss[idx][:],
                                    wqkv_sb[:, kc, m * 128:(m + 1) * 128],
                                    xn[:, kc, :],
                                    start=(kc == 0), stop=(kc == 7))
                        for idx, (n, xn) in enumerate(xns):
                            ps = pss[idx]
                            tok = slice(n * 512, (n + 1) * 512)
                            if m == 0:
                                nc.vector.tensor_copy(q_sb[:, tok], ps[:])
                            elif m == 1:
                                nc.vector.tensor_copy(k_sb[:, tok], ps[:])
                            else:
                                vtmp = vpool.tile([128, 512], f32)
                                nc.vector.tensor_copy(vtmp[:], ps[:])
                                b = n // NQ
                                for s in range(4):
                                    j = b * NK + (n % NQ) * 4 + s
                                    pt = vtp.tile([128, 128], f32)
                                    nc.tensor.transpose(
                                        pt[:],
                                        vtmp[:, s * 128:(s + 1) * 128],
                                        ident)
                                    nc.vector.tensor_copy(
                                        vboth[:, 0:64, j], pt[:, 0:64])
                                    nc.vector.tensor_copy(
                                        vboth[:, 65:129, j], pt[:, 64:128])

            # ---- phase 2 + 3: attention, then per-b proj + ReduceScatter ----
            with tc.tile_pool(name="s_psum", bufs=2, space="PSUM") as sp, \
                 tc.tile_pool(name="o_psum", bufs=4, space="PSUM") as op, \
                 tc.tile_pool(name="b_psum", bufs=1, space="PSUM") as bp, \
                 tc.tile_pool(name="pr_psum", bufs=1, space="PSUM") as prp, \
                 tc.tile_pool(name="attn_pool", bufs=6) as apool, \
                 tc.tile_pool(name="small_pool", bufs=2) as smp, \
                 tc.tile_pool(name="rb_pool", bufs=2) as rbp, \
                 tc.tile_pool(name="out_pool", bufs=2) as outp:
                for b in range(B):
                    for h in range(HL):
                        hs = slice(h * 64, (h + 1) * 64)
                        vcols = slice(65 * h, 65 * h + 65)
                        lrow = 64
                        drows = slice(0, 64)
                        po = [op.tile([65, 512], f32, tag="po", name="po")
                              for _ in range(NQ)]
                        for ki in range(NK):
                            qj0 = ki // 4 if causal else 0
                            ats = {}
                            for qj in range(qj0, NQ):
                                ps = sp.tile([128, 512], f32, tag="s")
                                nc.tensor.matmul(
                                    ps[:],
                                    k_sb[hs, b * T + ki * 128:
                                         b * T + (ki + 1) * 128],
                                    q_sb[hs, b * T + qj * 512:
                                         b * T + (qj + 1) * 512],
                                    start=True, stop=True)
                                at = apool.tile([128, 512], mdt, tag="at")
                                nc.scalar.activation(at[:], ps[:], Exp,
                                                     scale=SCALE)
                                if causal and qj == qj0:
                                    nc.vector.tensor_mul(
                                        at[:], at[:],
                                        cm_sb[:, (ki % 4) * 512:
                                              (ki % 4 + 1) * 512])
                                ats[qj] = at
                            for qj in range(qj0, NQ):
                                last = (4 * qj + 3) if causal else (NK - 1)
                                nc.tensor.matmul(
                                    po[qj][:], vboth[:, vcols, b * NK + ki],
                                    ats[qj][:],
                                    start=(ki == 0), stop=(ki == last))
                        # normalize: A = po[d] * (1/l) (l broadcast via PE)
                        lsb = smp.tile([1, 2048], f32)
                        for qj in range(NQ):
                            nc.vector.tensor_copy(
                                lsb[0:1, qj * 512:(qj + 1) * 512],
                                po[qj][lrow:lrow + 1, :])
                        rl = smp.tile([1, 2048], mdt)
                        nc.vector.reciprocal(rl[:], lsb[:])
                        rb = rbp.tile([64, 2048], f32)
                        for qj in range(NQ):
                            pb = bp.tile([64, 512], f32, tag="pb")
                            nc.tensor.matmul(
                                pb[:], ones_sb[:],
                                rl[0:1, qj * 512:(qj + 1) * 512],
                                start=True, stop=True)
                            nc.vector.tensor_copy(
                                rb[:, qj * 512:(qj + 1) * 512], pb[:])
                        for qj in range(NQ):
                            tok = slice(b * T + qj * 512,
                                        b * T + (qj + 1) * 512)
                            nc.vector.tensor_mul(
                                a_sb[hs, tok], po[qj][drows, :],
                                rb[:, qj * 512:(qj + 1) * 512])
                    # ---- proj partial + RS for this b's two token halves ----
                    for half in range(2):
                        c = b * 2 + half
                        t0 = b * T + half * 1024
                        for m in range(8):
                            for nn2 in range(2):
                                pr = prp.tile([128, 512], f32, tag="pr")
                                nc.tensor.matmul(
                                    pr[:],
                                    wp_sb[:, m * 128:(m + 1) * 128],
                                    a_sb[:, t0 + nn2 * 512:
                                         t0 + (nn2 + 1) * 512],
                                    start=True, stop=True)
                                pr_sb = outp.tile([128, 512], f32, tag="prs")
                                nc.vector.tensor_copy(pr_sb[:], pr[:])
                                nc.sync.dma_start(
                                    out=prt[c, m * 128:(m + 1) * 128,
                                            nn2 * 512:(nn2 + 1) * 512],
                                    in_=pr_sb[:])
                        nc.gpsimd.collective_compute(
                            "ReduceScatter", mybir.AluOpType.add,
                            replica_groups=[list(range(NCORES))],
                            ins=[prt[c].opt()], outs=[rso[c].opt()])
                        rs_sb = outp.tile([128, 1024], f32, tag="rs")
                        nc.sync.dma_start(out=rs_sb[:], in_=rso[c])
                        ot = outp.tile([128, 1024], f32, tag="ot")
                        nc.vector.tensor_scalar_add(ot[:], rs_sb[:],
                                                    bias_sb[:])
                        nc.sync.dma_start(
                            out=outT[:, c * 1024:(c + 1) * 1024], in_=ot[:])

    nc.compile()
    return nc


def _get_program(mode: str):
    if mode not in _cache:
        _cache[mode] = _build(mode)
    return _cache[mode]


def kernel(**inputs):
    from concourse.bass_utils import run_bass_kernel_spmd

    x = np.ascontiguousarray(np.asarray(inputs["x"], dtype=np.float32))
    mask = np.asarray(inputs["causal_mask"])
    Wqkv = np.ascontiguousarray(np.asarray(inputs["W_qkv"], dtype=np.float32))
    Wp = np.ascontiguousarray(np.asarray(inputs["W_proj"], dtype=np.float32))
    bp = np.asarray(inputs["b_proj"], dtype=np.float32)

    m2 = np.asarray(mask).reshape(T, T)
    if np.all(m2 != 0):
        mode = "none"
    else:
        tril = np.tril(np.ones((T, T), dtype=m2.dtype))
        if np.array_equal(m2, tril):
            mode = "causal"
        else:
            raise NotImplementedError("general mask not supported")

    nc = _get_program(mode)

    xT = np.ascontiguousarray(x.reshape(NT, C).T)  # [C, NT]

    # causal-mask tile patterns (valid iff p <= f - 128*j) + 128x128 identity
    p = np.arange(128)[:, None]
    f = np.arange(512)[None, :]
    cm = np.concatenate(
        [(p <= f - 128 * j).astype(np.float32) for j in range(4)]
        + [np.eye(128, dtype=np.float32)], axis=1)

    Wq = Wqkv[:, 0 * C:1 * C]
    Wk = Wqkv[:, 1 * C:2 * C]
    Wv = Wqkv[:, 2 * C:3 * C]

    in_maps = []
    for i in range(NCORES):
        hcols = slice(2 * i * D, (2 * i + 2) * D)  # this core's 2 heads
        wqkv_i = np.concatenate(
            [Wq[:, hcols], Wk[:, hcols], Wv[:, hcols]], axis=1)  # [C, 384]
        in_maps.append({
            "xT": xT,
            "wqkv": np.ascontiguousarray(wqkv_i),
            "wp": np.ascontiguousarray(Wp[i * 128:(i + 1) * 128, :]),
            "bias": np.ascontiguousarray(bp[i * 128:(i + 1) * 128]
                                         .reshape(128, 1)),
            "cmask": cm,
            "ones_in": np.ones((128, 64), dtype=np.float32),
        })

    res = run_bass_kernel_spmd(nc, in_maps, list(range(NCORES)))

    out = np.empty((NT, C), dtype=np.float32)
    for i in range(NCORES):
        out[:, i * 128:(i + 1) * 128] = res.results[i]["outT"].T
    return out.reshape(B, T, C)
